# revision 5
# baseline (speedup 1.0000x reference)
"""Trainium2 Bass kernel for nn_MultiHeadAttention_53815940219243.

Reference computation (single-head attention with full 1024-dim contraction):
    q = x @ Wq + bq; k = x @ Wk + bk; v = x @ Wv + bv        # [4096, 1024]
    scores = softmax(q @ k.T, axis=-1) / sqrt(64)            # [4096, 4096]
    z = scores @ v                                           # [4096, 1024]
    out = z @ Wo + bo                                        # [4096, 64]

Algebraic restructure (all weight-only products precomputed on host):
  * softmax is shift-invariant per row, so bk and the bq.(x Wq)-row term drop:
        S_eff[i,j] = x_i (Wq Wk^T) x_j^T + c_j,   c = x @ (Wk @ bq)
    With A = Wq Wk^T:  B = x A  (the only "Q/K projection" left), and the
    "K" operand of the score matmul is x itself -> no K projection and NO
    collectives at all (x is replicated; each core computes its 512 score
    columns).
  * exp(S + c) = exp(S) * exp(c): the per-key factor exp(c_j) is folded into
    the value rows, so the score phase is a pure matmul + Exp activation.
  * v@Wo is folded on host: vw = x @ (Wv Wo) + bv Wo  [4096, 64], and the
    softmax denominator comes free as a ones-column appended to vw:
        num[:, 0:64] = E^T @ (vw/8 * expc),  num[:, 64] = E^T @ expc = den/8?
    (the 1/8 = 1/sqrt(64) is folded into the vw columns only, so
     out = num[:,0:64] / num[:,64] + bo.)
  * No rank-1 bias matmuls anywhere (LDWEIGHTS-expensive): biases are added
    by DVE/ACT per-partition ops instead.

Dataflow per core (shard = 512 query rows; transposed score space, fp16
matmul operands, f32 PSUM accumulation; E stored bf16 since exp() can reach
e^60):
    B-proj: bt[dout, qi]   = sum_c a16[:,c,dout-tile]^T @ xq[:,c,:]    (fp16)
    S:      st[kj,  qi]    = sum_c xt[:,c,kj-tile]^T @ bt[:,c,:]       (fp16)
            et = Exp(st)                                               (bf16)
    V:      pv[kj, 0:66]   = sum_c xt[:,c,kj-tile]^T @ wvo_ext[:,c,:]  (fp16)
            expc = Exp(pv[:,65]); vw = (pv[:,0:65]+bvo_bc)*expc        (bf16)
    O:      o[h', qi]      = sum_jt vw[:,jt,:]^T @ et[:,jt,:]   (h'=65 rows)
            transpose 128-col tiles -> [qi, 65]; out = o[:,0:64]/o[:,64] + bo
"""

import numpy as np

N = 4096
D = 1024
H = 64
HP = H + 1      # 65: value cols + denominator ones-column
HE = H + 2      # 66: + c column (vw projection only)
NCORES = 8
NSH = N // NCORES   # 512 query rows per core
P = 128
DT = D // P         # 8 contraction chunks over the 1024 feature dim
JT = N // P         # 32 key tiles
IT = NSH // P       # 4 output row tiles per core

_CACHE = {}

# kept for test.py compatibility (all matmuls are fp16-operand regardless)
S_FP16 = True
PROJ_FP16 = True

# interleave the V-projection matmuls into the S loop so they share the
# already-loaded stationary operand (xt tile) with the S matmuls
SV_FUSED = True


def _build(upto="O", rep_a=1, rep_ag=1, rep_s=1, rep_u=1, rep_o=1,
           s_fp16=True, proj_fp16=True, sv_fused=None):
    if sv_fused is None:
        sv_fused = SV_FUSED
    import concourse.mybir as mybir
    import concourse.tile as tile
    from concourse import bacc
    from contextlib import ExitStack

    f32 = mybir.dt.float32
    bf16 = mybir.dt.bfloat16
    fp16 = mybir.dt.float16

    nc = bacc.Bacc("TRN2", target_bir_lowering=False, num_devices=NCORES)

    # ---- kernel I/O (per core; everything but xq is replicated) ----
    xt = nc.dram_tensor("xt", [P, DT, N], fp16, kind="ExternalInput")
    xq = nc.dram_tensor("xq", [P, DT, NSH], fp16, kind="ExternalInput")
    a16 = nc.dram_tensor("a16", [P, DT, D], fp16, kind="ExternalInput")
    wvo_e = nc.dram_tensor("wvo_e", [P, DT, HE], fp16, kind="ExternalInput")
    bvo_bc = nc.dram_tensor("bvo_bc", [P, HP], f32, kind="ExternalInput")
    bo_bc = nc.dram_tensor("bo_bc", [P, H], f32, kind="ExternalInput")
    ident = nc.dram_tensor("ident65", [HP, HP], f32, kind="ExternalInput")

    out = nc.dram_tensor("out", [NSH, H], f32, kind="ExternalOutput")

    with tile.TileContext(nc) as tc, ExitStack() as ctx:
        persist = ctx.enter_context(tc.tile_pool(name="persist", bufs=1))
        pp_big = ctx.enter_context(tc.tile_pool(name="pp_big", bufs=3, space="PSUM"))
        pp_sml = ctx.enter_context(tc.tile_pool(name="pp_sml", bufs=2, space="PSUM"))
        scratch = ctx.enter_context(tc.tile_pool(name="scratch", bufs=3))

        xt_sb = persist.tile([P, DT, N], fp16, tag="xt")
        xq_sb = persist.tile([P, DT, NSH], fp16, tag="xq")
        a_sb = persist.tile([P, DT, D], fp16, tag="a16")
        bt_sb = persist.tile([P, DT, NSH], fp16, tag="bt")
        et_sb = persist.tile([P, JT, NSH], bf16, tag="et")      # 32KB/part
        vw_sb = persist.tile([P, JT, HP], bf16, tag="vw")
        expc_sb = persist.tile([P, JT], f32, tag="expc")
        wvo_sb = persist.tile([P, DT, HE], fp16, tag="wvo")
        bvo_sb = persist.tile([P, HP], f32, tag="bvo")
        bo_sb = persist.tile([P, H], f32, tag="bo")
        id_sb = persist.tile([HP, HP], f32, tag="ident")

        # ---- input DMAs (issued up front; Tile tracks readiness) ----
        nc.sync.dma_start(out=xq_sb[:], in_=xq[:, :, :])
        for m in range(DT):
            nc.sync.dma_start(out=a_sb[:, :, m * P:(m + 1) * P],
                              in_=a16[:, :, m * P:(m + 1) * P])
        nc.sync.dma_start(out=wvo_sb[:], in_=wvo_e[:, :, :])
        nc.sync.dma_start(out=bvo_sb[:], in_=bvo_bc[:, :])
        nc.sync.dma_start(out=bo_sb[:], in_=bo_bc[:, :])
        nc.sync.dma_start(out=id_sb[:], in_=ident[:, :])
        for k in range(DT):
            nc.sync.dma_start(out=xt_sb[:, :, k * NSH:(k + 1) * NSH],
                              in_=xt[:, :, k * NSH:(k + 1) * NSH])

        # ---------------- phase B: bt = A^T-contraction with x shard ------
        for _r in range(rep_a):
            for m in range(DT):
                ps = pp_big.tile([P, NSH], f32, tag="ps")
                for c in range(DT):
                    nc.tensor.matmul(ps[:], a_sb[:, c, m * P:(m + 1) * P],
                                     xq_sb[:, c, :],
                                     start=(c == 0), stop=(c == DT - 1))
                nc.vector.tensor_copy(out=bt_sb[:, m, :], in_=ps[:])

        def v_tail(psv, jt):
            """expc = exp(c col); vw = (pv + bvo) * expc, stored bf16."""
            nc.scalar.activation(out=expc_sb[:, jt:jt + 1],
                                 in_=psv[:, HP:HE],
                                 func=mybir.ActivationFunctionType.Exp)
            t65 = scratch.tile([P, HP], f32, tag="t65")
            nc.vector.tensor_add(out=t65[:], in0=psv[:, 0:HP], in1=bvo_sb[:])
            nc.scalar.activation(out=vw_sb[:, jt, :], in_=t65[:],
                                 func=mybir.ActivationFunctionType.Copy,
                                 scale=expc_sb[:, jt:jt + 1])

        if sv_fused:
            # ------- phase S+V fused: V matmuls reuse the S stationary -----
            for _r in range(rep_s):
                for jt in range(JT):
                    ps = pp_big.tile([P, NSH], f32, tag="ps")
                    psv = pp_sml.tile([P, HE], f32, tag="psv")
                    for c in range(DT):
                        lhsT = xt_sb[:, c, jt * P:(jt + 1) * P]
                        nc.tensor.matmul(ps[:], lhsT, bt_sb[:, c, :],
                                         start=(c == 0), stop=(c == DT - 1))
                        nc.tensor.matmul(psv[:], lhsT, wvo_sb[:, c, :],
                                         start=(c == 0), stop=(c == DT - 1))
                    nc.scalar.activation(out=et_sb[:, jt, :], in_=ps[:],
                                         func=mybir.ActivationFunctionType.Exp)
                    v_tail(psv, jt)
        else:
            # ---------------- phase S: et = exp(x^T-tiles . bt) -----------
            for _r in range(rep_s):
                for jt in range(JT):
                    ps = pp_big.tile([P, NSH], f32, tag="ps")
                    for c in range(DT):
                        nc.tensor.matmul(ps[:], xt_sb[:, c, jt * P:(jt + 1) * P],
                                         bt_sb[:, c, :],
                                         start=(c == 0), stop=(c == DT - 1))
                    nc.scalar.activation(out=et_sb[:, jt, :], in_=ps[:],
                                         func=mybir.ActivationFunctionType.Exp)

            # -------- phase V: vw rows (+ ones col) scaled by exp(c) ------
            for _r in range(rep_ag):
                for jt in range(JT):
                    ps = pp_sml.tile([P, HE], f32, tag="psv")
                    for c in range(DT):
                        nc.tensor.matmul(ps[:], xt_sb[:, c, jt * P:(jt + 1) * P],
                                         wvo_sb[:, c, :],
                                         start=(c == 0), stop=(c == DT - 1))
                    v_tail(ps, jt)

        # -------- phase O: o = vw''^T @ E^T; transpose; divide; + bo ------
        for _r in range(rep_o):
            pso = pp_big.tile([HP, NSH], f32, tag="ps")
            for jt in range(JT):
                nc.tensor.matmul(pso[:], vw_sb[:, jt, :], et_sb[:, jt, :],
                                 start=(jt == 0), stop=(jt == JT - 1))
            osb = scratch.tile([HP, NSH], f32, tag="osb")
            nc.vector.tensor_copy(out=osb[:], in_=pso[:])
            for qt in range(IT):
                pst = pp_sml.tile([P, HP], f32, tag="pst")
                nc.tensor.transpose(pst[:], osb[:, qt * P:(qt + 1) * P], id_sb[:])
                r = scratch.tile([P, 1], f32, tag="rcp")
                nc.vector.reciprocal(out=r[:], in_=pst[:, H:HP])
                o_t = scratch.tile([P, H], f32, tag="osc")
                nc.vector.tensor_scalar_mul(out=o_t[:], in0=pst[:, 0:H],
                                            scalar1=r[:, 0:1])
                o_f = scratch.tile([P, H], f32, tag="ofin")
                nc.vector.tensor_add(out=o_f[:], in0=o_t[:], in1=bo_sb[:])
                nc.sync.dma_start(out=out[qt * P:(qt + 1) * P, :], in_=o_f[:])

    nc.finalize()
    return nc


def _prep_in_maps(x, Wq, bq, Wk, bk, Wv, bv, Wo, bo, proj_fp16=True):
    f32, f64 = np.float32, np.float64
    x = np.ascontiguousarray(x, dtype=f32)

    A = (np.asarray(Wq, f64) @ np.asarray(Wk, f64).T).astype(f32)
    wkbq = (np.asarray(Wk, f64) @ np.asarray(bq, f64)).astype(f32)
    wvo8 = ((np.asarray(Wv, f64) @ np.asarray(Wo, f64)) / 8.0).astype(f32)
    bvo8 = ((np.asarray(bv, f64) @ np.asarray(Wo, f64)) / 8.0).astype(f32)

    def dmaj(a):  # [1024(=c*128+p), F] -> [p, c, F] contiguous
        F = a.shape[1]
        return np.ascontiguousarray(
            a.reshape(DT, P, F).transpose(1, 0, 2)).astype(np.float16)

    xt16 = dmaj(x.T)                       # [128, 8, 4096]
    a16 = dmaj(A)                          # [128, 8, 1024]
    w66 = np.concatenate(
        [wvo8, np.zeros((D, 1), f32), wkbq[:, None]], axis=1)
    wvo_e = dmaj(w66)                      # [128, 8, 66]

    bvo_bc = np.ascontiguousarray(np.broadcast_to(
        np.concatenate([bvo8, np.array([1.0], f32)]), (P, HP))).astype(f32)
    bo_bc = np.ascontiguousarray(
        np.broadcast_to(np.asarray(bo, f32), (P, H))).astype(f32)
    ident = np.eye(HP, dtype=f32)

    shared = {
        "xt": xt16, "a16": a16, "wvo_e": wvo_e,
        "bvo_bc": bvo_bc, "bo_bc": bo_bc, "ident65": ident,
    }
    in_maps = []
    for c in range(NCORES):
        m = dict(shared)
        m["xq"] = np.ascontiguousarray(xt16[:, :, c * NSH:(c + 1) * NSH])
        in_maps.append(m)
    return in_maps


def kernel(x, Wq, bq, Wk, bk, Wv, bv, Wo, bo):
    from concourse.bass_utils import run_bass_kernel_spmd

    key = ("nc", S_FP16, PROJ_FP16)
    if key not in _CACHE:
        _CACHE[key] = _build(s_fp16=S_FP16, proj_fp16=PROJ_FP16)
    nc = _CACHE[key]

    in_maps = _prep_in_maps(x, Wq, bq, Wk, bk, Wv, bv, Wo, bo,
                            proj_fp16=PROJ_FP16)
    res = run_bass_kernel_spmd(nc, in_maps, core_ids=list(range(NCORES)))
    _CACHE["last_result"] = res
    return np.concatenate([res.results[c]["out"] for c in range(NCORES)], axis=0)
